# revision 9
# baseline (speedup 1.0000x reference)
"""Depthwise 3x3 conv (stride 1, SAME, depth_multiplier 1) on 8 trn2 NeuronCores.

Input  x [32, 112, 112, 192] f32, w [3, 3, 1, 192] f32, b [192] f32.
Output [32, 112, 112, 192] f32.

Strategy (pure data parallelism, batch sharded 4 images/core):
  SBUF layout: partitions = h (112 rows), free = (w, c) with one zero guard
  w-slot on each side (114 x 192 elements, bf16).
  - ScalarE casts fp32 input chunks to bf16 into the image tile.
  - VectorE computes the 9 tap products per window: prod_t = x(w+dw) * W[t,c]
    (tensor_tensor mult vs a broadcast weight tile, bf16 2x mode).
  - TensorE applies the h-shifts and sums all taps + bias into PSUM (fp32):
    psum[:, chunk] += S_dh.T @ prod_t; S matrices are 0/1 shift matrices that
    also implement SAME padding at h edges; a 10th matmul adds the bias row.
  - ScalarE evacuates PSUM -> SBUF fp32; HWDGE DMA writes NHWC output.
"""
import numpy as np
import ml_dtypes

import concourse.bacc as bacc
import concourse.mybir as mybir
from concourse.bass_utils import run_bass_kernel_spmd
from concourse.tile import TileContext

F32 = mybir.dt.float32
BF16 = mybir.dt.bfloat16

B, H, W, C = 32, 112, 112, 192
N_CORES = 8
B_SH = B // N_CORES          # images per core
WWIN = 8                     # w columns per window
PCH = 512                    # PSUM chunk (one bank of fp32)


class _Geom:
    def __init__(self, h=H, w=W, c=C):
        self.h, self.w, self.c = h, w, c
        self.wwin = WWIN
        self.nwin = w // self.wwin
        self.wfree = self.wwin * c
        self.nch = self.wfree // PCH
        self.wg = w + 2
        self.xfree = self.wg * c


def _alloc_tiles(nc, tc, g, cpool, xpool, ppool, wb_compress=False):
    if wb_compress:
        wb = cpool.tile([128, 9 * g.c + g.wfree], BF16, tag="wb", name="wb")
    else:
        wb = cpool.tile([128, 10 * g.wfree], BF16, tag="wb", name="wb")
    sm = cpool.tile([128, 4 * 128], BF16, tag="sm", name="sm")
    xts = [xpool.tile([g.h, g.xfree], BF16, tag=f"x{i}", name=f"x{i}")
           for i in range(2)]
    for xt in xts:
        nc.vector.memset(xt[:, 0:g.c], 0.0)
        nc.vector.memset(xt[:, (g.wg - 1) * g.c:g.wg * g.c], 0.0)
    prods = [[ppool.tile([128, g.wfree], BF16, tag=f"p{s}_{t}",
                         name=f"p{s}_{t}") for t in range(9)]
             for s in range(2)]
    for s in range(2):
        for t in range(9):
            nc.vector.memset(prods[s][t][:, :], 0.0)
    return wb, sm, xts, prods


def _emit_image(nc, g, wb, sm, xt, prods, spool, opool, pspool, x_img, y_img,
                skip=(), opts=None):
    """Emit load+compute+store for one image. x_img/y_img: [h, w, c] DRAM APs."""
    opts = opts or {}
    lc = opts.get("load_chunk", 1)        # windows per load DMA
    mm_order = opts.get("mm_order", "dh")
    wbc = opts.get("wb_compress", True)
    c, wwin, wfree, nch = g.c, g.wwin, g.wfree, g.nch
    for wd in range(0, g.nwin, lc):
        if "load" in skip:
            break
        w0 = wd * wwin
        stg = spool.tile([g.h, wfree * lc], F32, tag="stg", name="stg")
        nc.sync.dma_start(
            out=stg[:, :],
            in_=x_img[:, w0:w0 + wwin * lc, :].rearrange("h w c -> h (w c)"))
        nc.scalar.activation(
            xt[:, (1 + w0) * c:(1 + w0 + wwin * lc) * c], stg[:, :],
            mybir.ActivationFunctionType.Copy)
    for wd in range(g.nwin):
        w0 = wd * wwin
        pset = prods[wd % 2]
        for dh in range(3):
            if "mult" in skip:
                break
            for dw in range(3):
                t = dh * 3 + dw
                off = (w0 + dw) * c
                if wbc:
                    nc.vector.tensor_mul(
                        pset[t][:g.h, :].rearrange("h (w c) -> h w c", c=c),
                        xt[:, off:off + wfree].rearrange("h (w c) -> h w c", c=c),
                        wb[:g.h, t * c:(t + 1) * c].unsqueeze(1)
                        .broadcast_to([g.h, wwin, c]))
                else:
                    nc.vector.tensor_mul(
                        pset[t][:g.h, :],
                        xt[:, off:off + wfree],
                        wb[:g.h, t * wfree:(t + 1) * wfree])
        if "pe" in skip:
            continue
        ps = pspool.tile([128, wfree], F32, tag="ps", name="ps")
        def _sl(ch):
            return slice(ch * PCH, (ch + 1) * PCH)
        if mm_order == "ch":
            for ch in range(nch):
                sl = _sl(ch)
                for dh in range(3):
                    for dw in range(3):
                        t = dh * 3 + dw
                        nc.tensor.matmul(
                            ps[:, sl], sm[:, dh * 128:(dh + 1) * 128],
                            pset[t][:, sl],
                            start=(dh == 0 and dw == 0), stop=False)
                bias0 = 9 * c if wbc else 9 * wfree
                nc.tensor.matmul(
                    ps[:, sl], sm[:, 3 * 128:4 * 128],
                    wb[:, bias0 + ch * PCH:bias0 + (ch + 1) * PCH],
                    start=False, stop=True)
        else:  # dh-major: long same-stationary runs
            for dh in range(3):
                for ch in range(nch):
                    sl = _sl(ch)
                    for dw in range(3):
                        t = dh * 3 + dw
                        nc.tensor.matmul(
                            ps[:, sl], sm[:, dh * 128:(dh + 1) * 128],
                            pset[t][:, sl],
                            start=(dh == 0 and dw == 0), stop=False)
            bias0 = 9 * c if wbc else 9 * wfree
            for ch in range(nch):
                sl = _sl(ch)
                nc.tensor.matmul(
                    ps[:, sl], sm[:, 3 * 128:4 * 128],
                    wb[:, bias0 + ch * PCH:bias0 + (ch + 1) * PCH],
                    start=False, stop=True)
        outc = opool.tile([g.h, wfree], F32, tag="outc", name="outc")
        nc.scalar.activation(outc[:, :], ps[:g.h, :],
                             mybir.ActivationFunctionType.Copy)
        nc.sync.dma_start(
            out=y_img[:, w0:w0 + wwin, :].rearrange("h w c -> h (w c)"),
            in_=outc[:, :])


def _pools(nc, tc):
    return (
        tc.tile_pool(name="const", bufs=1),
        tc.tile_pool(name="xp", bufs=1),
        tc.tile_pool(name="prodp", bufs=1),
        tc.tile_pool(name="stg", bufs=2),
        tc.tile_pool(name="outp", bufs=3),
        tc.tile_pool(name="psum", bufs=2, space="PSUM"),
    )


def _build_module(b_sh=B_SH, h=H, w=W, c=C, opts=None):
    g = _Geom(h, w, c)
    opts = dict(_DEFAULT_OPTS, **(opts or {}))
    nc = bacc.Bacc("TRN2")
    x = nc.dram_tensor("x", [b_sh, h, w, c], F32, kind="ExternalInput")
    nwb = 9 * c + g.wfree if opts.get("wb_compress") else 10 * g.wfree
    wbias = nc.dram_tensor("wbias", [128, nwb], BF16,
                           kind="ExternalInput")
    smats = nc.dram_tensor("smats", [128, 4 * 128], BF16, kind="ExternalInput")
    y = nc.dram_tensor("y", [b_sh, h, w, c], F32, kind="ExternalOutput")

    with TileContext(nc) as tc:
        with (
            tc.tile_pool(name="const", bufs=1) as cpool,
            tc.tile_pool(name="xp", bufs=1) as xpool,
            tc.tile_pool(name="prodp", bufs=1) as ppool,
            tc.tile_pool(name="stg", bufs=opts.get("stg_bufs", 2)) as spool,
            tc.tile_pool(name="outp", bufs=opts.get("out_bufs", 3)) as opool,
            tc.tile_pool(name="psum", bufs=2, space="PSUM") as pspool,
        ):
            wb, sm, xts, prods = _alloc_tiles(nc, tc, g, cpool, xpool, ppool,
                                              opts.get("wb_compress", False))
            nc.sync.dma_start(out=wb[:, :], in_=wbias[:, :])
            nc.sync.dma_start(out=sm[:, :], in_=smats[:, :])
            for img in range(b_sh):
                _emit_image(nc, g, wb, sm, xts[img % 2], prods,
                            spool, opool, pspool, x[img], y[img], opts=opts)
    nc.compile()
    return nc


def _build_timing_module(h=H, w=W, c=C, iters=8, skip=(), opts=None):
    """Same per-image pipeline in a HW loop over internal DRAM tensors.

    One loop iteration = 2 image passes (ping-pong tiles). No host IO.
    """
    g = _Geom(h, w, c)
    opts = dict(_DEFAULT_OPTS, **(opts or {}))
    nc = bacc.Bacc("TRN2")
    x = nc.dram_tensor("xg", [2, h, w, c], F32)
    y = nc.dram_tensor("yg", [2, h, w, c], F32)
    yo = nc.dram_tensor("yo", [1, 8], F32, kind="ExternalOutput")

    with TileContext(nc) as tc:
        with (
            tc.tile_pool(name="const", bufs=1) as cpool,
            tc.tile_pool(name="xp", bufs=1) as xpool,
            tc.tile_pool(name="prodp", bufs=1) as ppool,
            tc.tile_pool(name="stg", bufs=opts.get("stg_bufs", 2)) as spool,
            tc.tile_pool(name="outp", bufs=opts.get("out_bufs", 3)) as opool,
            tc.tile_pool(name="psum", bufs=2, space="PSUM") as pspool,
        ):
            wb, sm, xts, prods = _alloc_tiles(nc, tc, g, cpool, xpool, ppool,
                                              opts.get("wb_compress", False))
            nc.vector.memset(wb[:, :], 0.01)
            nc.vector.memset(sm[:, :], 0.0)
            # zero the source so bf16 garbage can't produce NaNs
            zt = spool.tile([g.h, g.wfree], F32, tag="stg", name="zt")
            nc.vector.memset(zt[:, :], 0.5)
            for img in range(2):
                for wd in range(g.nwin):
                    nc.sync.dma_start(
                        out=x[img, :, wd * g.wwin:(wd + 1) * g.wwin, :]
                        .rearrange("h w c -> h (w c)"),
                        in_=zt[:, :])
            with tc.For_i(0, iters) as _:
                for img in range(2):
                    _emit_image(nc, g, wb, sm, xts[img], prods,
                                spool, opool, pspool, x[img], y[img], skip=skip,
                                opts=opts)
            of = opool.tile([1, 8], F32, tag="outc", name="of")
            nc.vector.memset(of[:, :], 0.0)
            nc.sync.dma_start(out=yo[:, :], in_=of[:1, :8])
    nc.compile()
    return nc


def _host_consts(wk, bk, h=H, w=W, c=C, wb_compress=False):
    """wk [3,3,1,192] f32, bk [192] f32 -> (wbias bf16, smats [128,512] bf16)."""
    g = _Geom(h, w, c)
    wfree = g.wfree
    if wb_compress:
        wb = np.zeros((128, 9 * c + wfree), np.float32)
        for dh in range(3):
            for dw in range(3):
                t = dh * 3 + dw
                wb[:, t * c:(t + 1) * c] = wk[dh, dw, 0][None, :]
        wb[:, 9 * c:9 * c + wfree] = np.tile(bk, g.wwin)[None, :]
    else:
        wb = np.zeros((128, 10 * wfree), np.float32)
        for dh in range(3):
            for dw in range(3):
                t = dh * 3 + dw
                pat = np.tile(wk[dh, dw, 0], g.wwin)
                wb[:, t * wfree:(t + 1) * wfree] = pat[None, :]
        wb[:, 9 * wfree:10 * wfree] = np.tile(bk, g.wwin)[None, :]

    sm = np.zeros((128, 4 * 128), np.float32)
    for i, dh in enumerate((-1, 0, 1)):
        for m in range(h):
            k = m + dh
            if 0 <= k < h:
                sm[k, i * 128 + m] = 1.0
    sm[0, 3 * 128:3 * 128 + h] = 1.0  # bias selector row
    return (wb.astype(ml_dtypes.bfloat16), sm.astype(ml_dtypes.bfloat16))


_DEFAULT_OPTS = dict(mm_order="dh", wb_compress=True)

_NC_CACHE = {}


def kernel(x, w, b):
    x = np.ascontiguousarray(np.asarray(x, dtype=np.float32))
    wk = np.asarray(w, dtype=np.float32)
    bk = np.asarray(b, dtype=np.float32)
    assert x.shape == (B, H, W, C), x.shape

    if "nc" not in _NC_CACHE:
        _NC_CACHE["nc"] = _build_module(opts=_DEFAULT_OPTS)
    nc = _NC_CACHE["nc"]

    wbias, smats = _host_consts(wk, bk, wb_compress=_DEFAULT_OPTS["wb_compress"])
    in_maps = []
    for core in range(N_CORES):
        sh = x[core * B_SH:(core + 1) * B_SH]
        in_maps.append({"x": np.ascontiguousarray(sh), "wbias": wbias,
                        "smats": smats})
    res = run_bass_kernel_spmd(nc, in_maps, core_ids=list(range(N_CORES)))
    out = np.empty((B, H, W, C), np.float32)
    for core in range(N_CORES):
        out[core * B_SH:(core + 1) * B_SH] = res.results[core]["y"]
    return out
